# revision 4
# baseline (speedup 1.0000x reference)
"""GP prediction kernel for Trainium2 (8 NeuronCores, data-parallel over batch).

Computes z_pred[b, p, d] = sum_c k_mult[p, c] * z_enc[b, c, d] where k_mult
is the [64, 448] GP weight matrix k_pred.T @ inv(cov + sigma*I). k_mult
depends only on compile-time constants, so it is precomputed on host; the
device work is a batched [64,448] @ [448,1024] matmul, sharded 8 batches
per core.

Mixed precision against the 2e-2 correctness gate: the contraction is
split by column energy of k_mult. The 192 highest-energy context steps
travel in fp16 (two K-tiles: 128 + 64 rows). The 256 lowest-energy steps
(2.6% of the weight energy) travel as fp8-e4m3 with a per-column balanced
scale g_c = sqrt(max|km_c|) folded into both operands (km/g, z*g) to keep
values out of fp8's subnormal range; they are consumed by a single
DoubleRow matmul (2 contraction rows/cycle). End-to-end error ~6e-3.
This cuts HBM traffic from 8.45 to 6.2 MB/core and tensor-engine passes
from 4 to 3 per output tile - both engines sit at ~17-18us, down from
~23-24us for pure fp16.
"""
import numpy as np
from contextlib import ExitStack

import concourse.bacc as bacc
import concourse.tile as tile
from concourse import mybir
from concourse.bass_utils import run_bass_kernel_spmd

# Problem constants (hardcoded per harness contract).
B, T, D = 64, 512, 1024
P = 64                 # N_PREDICTORS
C = T - P              # 448 context timesteps
L, SIGMA, TIMESCALE = 0.01, 0.01, 0.3
N_CORES = 8
BPC = B // N_CORES     # batches per core

N8 = 256               # low-energy columns in fp8 (one DoubleRow unit)
N16 = C - N8           # high-energy columns in fp16 (128 + 64 K-tiles)

F8 = mybir.dt.np(mybir.dt.float8e4)   # ml_dtypes.float8_e4m3


def _k_mult() -> np.ndarray:
    """[P, C] GP weight matrix, solved in float64 on host."""
    t = np.linspace(0.0, 1.0, T)
    t_in = t[:C] * TIMESCALE
    t_pred = t[C:] * TIMESCALE

    def rbf_np(x, y):
        d = x[:, None] - y[None, :]
        return np.exp(-0.5 * d * d / L)

    cov = rbf_np(t_in, t_in) + np.eye(C) * SIGMA
    return np.linalg.solve(cov, rbf_np(t_in, t_pred)).T   # [P, C] float64


def _prep_constants():
    km = _k_mult()
    energy = (km * km).sum(axis=0)
    order = np.argsort(energy)
    cols8 = np.sort(order[:N8])       # fp8 columns, natural order
    cols16 = np.sort(order[N8:])      # fp16 columns

    # fp16 weights: [128, 2*P]; block 0 = cols16[0:128], block 1 (p<64) =
    # cols16[128:192]
    km16 = np.zeros((128, 2 * P), np.float16)
    km16[:, :P] = km[:, cols16[:128]].T
    km16[:64, P:] = km[:, cols16[128:]].T

    # fp8 weights with balanced per-column scale, [128, 2, P]:
    # subtile q holds cols8[q*128 + p]
    g = np.sqrt(np.abs(km[:, cols8]).max(axis=0))          # [N8]
    km8 = (km[:, cols8] / g).astype(F8)                    # [P, N8]
    km8_dev = np.zeros((128, 2, P), F8)
    for q in range(2):
        km8_dev[:, q, :] = km8[:, q * 128 : (q + 1) * 128].T
    return cols16, cols8, g.astype(np.float32), km16, km8_dev.reshape(128, 2 * P)


COLS16, COLS8, G8, KM16_DEV, KM8_DEV = _prep_constants()

_NC = None


def _build():
    nc = bacc.Bacc()
    # fp16 part, per batch: block of [192, D] rows (0:128 = K-tile 0,
    # 128:192 = K-tile 1)
    z16 = nc.dram_tensor("z16", [BPC * N16, D], mybir.dt.float16,
                         kind="ExternalInput")
    # fp8 part, per batch: [128, 2*D] (partition p, subtile-major bytes)
    z8 = nc.dram_tensor("z8", [BPC * 128, 2 * D], mybir.dt.float8e4,
                        kind="ExternalInput")
    km16 = nc.dram_tensor("km16", [128, 2 * P], mybir.dt.float16,
                          kind="ExternalInput")
    km8 = nc.dram_tensor("km8", [128, 2 * P], mybir.dt.float8e4,
                         kind="ExternalInput")
    out = nc.dram_tensor("out", [BPC * P, D], mybir.dt.float16,
                         kind="ExternalOutput")

    with tile.TileContext(nc) as tc, ExitStack() as ctx:
        kpool = ctx.enter_context(tc.tile_pool(name="km", bufs=1))
        z16pool = ctx.enter_context(tc.tile_pool(name="z16", bufs=4))
        zj1pool = ctx.enter_context(tc.tile_pool(name="zj1", bufs=4))
        z8pool = ctx.enter_context(tc.tile_pool(name="z8", bufs=4))
        opool = ctx.enter_context(tc.tile_pool(name="o", bufs=4))
        ppool = ctx.enter_context(tc.tile_pool(name="ps", bufs=8, space="PSUM"))

        km16_sb = kpool.tile([128, 2 * P], mybir.dt.float16)
        km8_sb = kpool.tile([128, 2, P], mybir.dt.float8e4)
        nc.scalar.dma_start(km16_sb[:, :], km16[:, :])
        nc.sync.dma_start(km8_sb[:, :, :], km8[:, :])

        for b in range(BPC):
            e0 = nc.sync if b % 2 == 0 else nc.scalar
            e1 = nc.scalar if b % 2 == 0 else nc.sync

            zj0 = z16pool.tile([128, D], mybir.dt.float16, name=f"zj0_{b}",
                               tag="zj0")
            e0.dma_start(zj0[:, :], z16[b * N16 : b * N16 + 128, :])
            zj1 = zj1pool.tile([64, D], mybir.dt.float16, name=f"zj1_{b}",
                               tag="zj1")
            e0.dma_start(zj1[:, :], z16[b * N16 + 128 : (b + 1) * N16, :])
            z8t = z8pool.tile([128, 2, D], mybir.dt.float8e4, name=f"z8_{b}",
                              tag="z8")
            e1.dma_start(z8t[:, :, :], z8[b * 128 : (b + 1) * 128, :])

            out_sb = opool.tile([P, D], mybir.dt.float16, name=f"o_{b}",
                                tag="o")
            for n in range(2):
                ps = ppool.tile([P, 512], mybir.dt.float32)
                nc.tensor.matmul(
                    ps[:, :], km16_sb[:, :P],
                    zj0[:, n * 512 : (n + 1) * 512],
                    start=True, stop=False,
                )
                nc.tensor.matmul(
                    ps[:, :], km16_sb[:64, P : 2 * P],
                    zj1[:, n * 512 : (n + 1) * 512],
                    start=False, stop=False,
                )
                nc.tensor.matmul(
                    ps[:, :], km8_sb[:, :, :],
                    z8t[:, :, n * 512 : (n + 1) * 512],
                    start=False, stop=True,
                    perf_mode=mybir.MatmulPerfMode.DoubleRow,
                )
                nc.vector.tensor_copy(
                    out_sb[:, n * 512 : (n + 1) * 512], ps[:, :]
                )
            eo = nc.sync if b % 2 == 1 else nc.scalar
            eo.dma_start(out[b * P : (b + 1) * P, :], out_sb[:, :])

    nc.finalize()
    return nc


def kernel(z_enc: np.ndarray, _trace: bool = False):
    global _NC
    z_enc = np.asarray(z_enc, dtype=np.float32)
    if _NC is None:
        _NC = _build()

    zc = z_enc[:, :C, :]
    z16 = np.ascontiguousarray(zc[:, COLS16, :]).astype(np.float16)  # [B,192,D]
    z8f = np.ascontiguousarray(zc[:, COLS8, :]) * G8[None, :, None]  # [B,256,D]
    # [B, 256, D] -> [B, 128, 2, D] so partition p holds subtiles q=0,1
    z8 = z8f.reshape(B, 2, 128, D).transpose(0, 2, 1, 3).astype(F8)

    in_maps = [
        {
            "z16": z16[i * BPC : (i + 1) * BPC].reshape(BPC * N16, D),
            "z8": np.ascontiguousarray(
                z8[i * BPC : (i + 1) * BPC]).reshape(BPC * 128, 2 * D),
            "km16": KM16_DEV,
            "km8": KM8_DEV,
        }
        for i in range(N_CORES)
    ]

    res = run_bass_kernel_spmd(_NC, in_maps, core_ids=list(range(N_CORES)),
                               trace=_trace)
    out = np.concatenate(
        [r["out"].reshape(BPC, P, D) for r in res.results], axis=0
    ).astype(np.float32)
    if _trace:
        return out, res
    return out


# revision 5
# speedup vs baseline: 1.0127x; 1.0127x over previous
"""GP prediction kernel for Trainium2 (8 NeuronCores, data-parallel over batch).

Computes z_pred[b, p, d] = sum_c k_mult[p, c] * z_enc[b, c, d] where k_mult
is the [64, 448] GP weight matrix k_pred.T @ inv(cov + sigma*I). k_mult
depends only on compile-time constants, so it is precomputed on host; the
device work is a batched [64,448] @ [448,1024] matmul, sharded 8 batches
per core.

Mixed precision against the 2e-2 correctness gate: the contraction is
split by column energy of k_mult. The 192 highest-energy context steps
travel in fp16 (two K-tiles: 128 + 64 rows). The 256 lowest-energy steps
(2.6% of the weight energy) travel as fp8-e4m3 with a per-column balanced
scale g_c = sqrt(max|km_c|) folded into both operands (km/g, z*g) to keep
values out of fp8's subnormal range; they are consumed by a single
DoubleRow matmul (2 contraction rows/cycle). End-to-end error ~6e-3.
This cuts HBM traffic from 8.45 to 6.2 MB/core and tensor-engine passes
from 4 to 3 per output tile - both engines sit at ~17-18us, down from
~23-24us for pure fp16.
"""
import numpy as np
from contextlib import ExitStack

import concourse.bacc as bacc
import concourse.tile as tile
from concourse import mybir
from concourse.bass_utils import run_bass_kernel_spmd

# Problem constants (hardcoded per harness contract).
B, T, D = 64, 512, 1024
P = 64                 # N_PREDICTORS
C = T - P              # 448 context timesteps
L, SIGMA, TIMESCALE = 0.01, 0.01, 0.3
N_CORES = 8
BPC = B // N_CORES     # batches per core

N8 = 256               # low-energy columns in fp8 (one DoubleRow unit)
N16 = C - N8           # high-energy columns in fp16 (128 + 64 K-tiles)

F8 = mybir.dt.np(mybir.dt.float8e4)   # ml_dtypes.float8_e4m3


def _k_mult() -> np.ndarray:
    """[P, C] GP weight matrix, solved in float64 on host."""
    t = np.linspace(0.0, 1.0, T)
    t_in = t[:C] * TIMESCALE
    t_pred = t[C:] * TIMESCALE

    def rbf_np(x, y):
        d = x[:, None] - y[None, :]
        return np.exp(-0.5 * d * d / L)

    cov = rbf_np(t_in, t_in) + np.eye(C) * SIGMA
    return np.linalg.solve(cov, rbf_np(t_in, t_pred)).T   # [P, C] float64


def _prep_constants():
    km = _k_mult()
    energy = (km * km).sum(axis=0)
    order = np.argsort(energy)
    cols8 = np.sort(order[:N8])       # fp8 columns, natural order
    cols16 = np.sort(order[N8:])      # fp16 columns

    # fp16 weights: [128, 2*P]; block 0 = cols16[0:128], block 1 (p<64) =
    # cols16[128:192]
    km16 = np.zeros((128, 2 * P), np.float16)
    km16[:, :P] = km[:, cols16[:128]].T
    km16[:64, P:] = km[:, cols16[128:]].T

    # fp8 weights with balanced per-column scale, [128, 2, P]:
    # subtile q holds cols8[q*128 + p]
    g = np.sqrt(np.abs(km[:, cols8]).max(axis=0))          # [N8]
    km8 = (km[:, cols8] / g).astype(F8)                    # [P, N8]
    km8_dev = np.zeros((128, 2, P), F8)
    for q in range(2):
        km8_dev[:, q, :] = km8[:, q * 128 : (q + 1) * 128].T
    return cols16, cols8, g.astype(np.float32), km16, km8_dev.reshape(128, 2 * P)


COLS16, COLS8, G8, KM16_DEV, KM8_DEV = _prep_constants()

_NC = None


def _build():
    nc = bacc.Bacc()
    # fp16 part, per batch: block of [192, D] rows (0:128 = K-tile 0,
    # 128:192 = K-tile 1)
    z16 = nc.dram_tensor("z16", [BPC * N16, D], mybir.dt.float16,
                         kind="ExternalInput")
    # fp8 part, per batch: [128, 2*D] (partition p, subtile-major bytes)
    z8 = nc.dram_tensor("z8", [BPC * 128, 2 * D], mybir.dt.float8e4,
                        kind="ExternalInput")
    km16 = nc.dram_tensor("km16", [128, 2 * P], mybir.dt.float16,
                          kind="ExternalInput")
    km8 = nc.dram_tensor("km8", [128, 2 * P], mybir.dt.float8e4,
                         kind="ExternalInput")
    out = nc.dram_tensor("out", [BPC * P, D], mybir.dt.float16,
                         kind="ExternalOutput")

    with tile.TileContext(nc) as tc, ExitStack() as ctx:
        kpool = ctx.enter_context(tc.tile_pool(name="km", bufs=1))
        z16pool = ctx.enter_context(tc.tile_pool(name="z16", bufs=4))
        zj1pool = ctx.enter_context(tc.tile_pool(name="zj1", bufs=4))
        z8pool = ctx.enter_context(tc.tile_pool(name="z8", bufs=4))
        opool = ctx.enter_context(tc.tile_pool(name="o", bufs=4))
        ppool = ctx.enter_context(tc.tile_pool(name="ps", bufs=8, space="PSUM"))

        km16_sb = kpool.tile([128, 2 * P], mybir.dt.float16)
        km8_sb = kpool.tile([128, 2, P], mybir.dt.float8e4)
        nc.scalar.dma_start(km16_sb[:, :], km16[:, :])
        nc.sync.dma_start(km8_sb[:, :, :], km8[:, :])

        for b in range(BPC):
            e0 = nc.sync if b % 2 == 0 else nc.scalar
            e1 = nc.scalar if b % 2 == 0 else nc.sync

            zj0 = z16pool.tile([128, D], mybir.dt.float16, name=f"zj0_{b}",
                               tag="zj0")
            e0.dma_start(zj0[:, :], z16[b * N16 : b * N16 + 128, :])
            zj1 = zj1pool.tile([64, D], mybir.dt.float16, name=f"zj1_{b}",
                               tag="zj1")
            e0.dma_start(zj1[:, :], z16[b * N16 + 128 : (b + 1) * N16, :])
            z8t = z8pool.tile([128, 2, D], mybir.dt.float8e4, name=f"z8_{b}",
                              tag="z8")
            e1.dma_start(z8t[:, :, :], z8[b * 128 : (b + 1) * 128, :])

            out_sb = opool.tile([P, D], mybir.dt.float16, name=f"o_{b}",
                                tag="o")
            for n in range(2):
                ps = ppool.tile([P, 512], mybir.dt.float32)
                nc.tensor.matmul(
                    ps[:, :], km16_sb[:, :P],
                    zj0[:, n * 512 : (n + 1) * 512],
                    start=True, stop=False,
                )
                nc.tensor.matmul(
                    ps[:, :], km16_sb[:64, P : 2 * P],
                    zj1[:, n * 512 : (n + 1) * 512],
                    start=False, stop=False,
                )
                for q in range(2):
                    nc.tensor.matmul(
                        ps[:, :], km8_sb[:, q, :],
                        z8t[:, q, n * 512 : (n + 1) * 512],
                        start=False, stop=(q == 1),
                    )
                nc.vector.tensor_copy(
                    out_sb[:, n * 512 : (n + 1) * 512], ps[:, :]
                )
            eo = nc.sync if b % 2 == 1 else nc.scalar
            eo.dma_start(out[b * P : (b + 1) * P, :], out_sb[:, :])

    nc.finalize()
    return nc


def kernel(z_enc: np.ndarray, _trace: bool = False):
    global _NC
    z_enc = np.asarray(z_enc, dtype=np.float32)
    if _NC is None:
        _NC = _build()

    zc = z_enc[:, :C, :]
    z16 = np.ascontiguousarray(zc[:, COLS16, :]).astype(np.float16)  # [B,192,D]
    z8f = np.ascontiguousarray(zc[:, COLS8, :]) * G8[None, :, None]  # [B,256,D]
    # [B, 256, D] -> [B, 128, 2, D] so partition p holds subtiles q=0,1
    z8 = z8f.reshape(B, 2, 128, D).transpose(0, 2, 1, 3).astype(F8)

    in_maps = [
        {
            "z16": z16[i * BPC : (i + 1) * BPC].reshape(BPC * N16, D),
            "z8": np.ascontiguousarray(
                z8[i * BPC : (i + 1) * BPC]).reshape(BPC * 128, 2 * D),
            "km16": KM16_DEV,
            "km8": KM8_DEV,
        }
        for i in range(N_CORES)
    ]

    res = run_bass_kernel_spmd(_NC, in_maps, core_ids=list(range(N_CORES)),
                               trace=_trace)
    out = np.concatenate(
        [r["out"].reshape(BPC, P, D) for r in res.results], axis=0
    ).astype(np.float32)
    if _trace:
        return out, res
    return out


# revision 10
# speedup vs baseline: 1.0333x; 1.0204x over previous
"""GP prediction kernel for Trainium2 (8 NeuronCores, data-parallel over batch).

Computes z_pred[b, p, d] = sum_c k_mult[p, c] * z_enc[b, c, d] where k_mult
is the [64, 448] GP weight matrix k_pred.T @ inv(cov + sigma*I). k_mult
depends only on compile-time constants, so it is precomputed on host; the
device work is a batched [64,448] @ [448,1024] matmul, sharded 8 batches
per core.

Mixed precision against the 2e-2 correctness gate: the contraction is
split by column energy of k_mult. The 192 highest-energy context steps
travel in fp16 (K-tiles of 128 + 64 rows); the 256 lowest-energy steps
(2.6% of weight energy) travel as fp8-e4m3 with a per-column balanced
scale g_c = sqrt(max|km_c|) folded into both operands, consumed by two
plain fp8 matmuls (NOT DoubleRow - that mode pins the PE power governor
to the half-speed p-state for the whole kernel). End-to-end error ~6e-3.

Batches are packed in pairs per SBUF tile to halve the DMA instruction
count, and a burst of garbage warm-up matmuls runs while the first real
tiles are still in flight so the PE power governor's full-speed grant
(which needs sustained PE activity) arrives before the real matmuls do.
"""
import numpy as np
from contextlib import ExitStack

import concourse.bacc as bacc
import concourse.tile as tile
from concourse import mybir
from concourse.bass_utils import run_bass_kernel_spmd

# Problem constants (hardcoded per harness contract).
B, T, D = 64, 512, 1024
P = 64                 # N_PREDICTORS
C = T - P              # 448 context timesteps
L, SIGMA, TIMESCALE = 0.01, 0.01, 0.3
N_CORES = 8
BPC = B // N_CORES     # batches per core
NPAIR = BPC // 2       # batch pairs per core

N8 = 256               # low-energy columns in fp8 (2 K-tiles of 128)
N16 = C - N8           # high-energy columns in fp16 (128 + 64 K-tiles)
NWARM = 12             # garbage matmuls to pull the PE p-state grant early

F8 = mybir.dt.np(mybir.dt.float8e4)   # ml_dtypes.float8_e4m3


def _k_mult() -> np.ndarray:
    """[P, C] GP weight matrix, solved in float64 on host."""
    t = np.linspace(0.0, 1.0, T)
    t_in = t[:C] * TIMESCALE
    t_pred = t[C:] * TIMESCALE

    def rbf_np(x, y):
        d = x[:, None] - y[None, :]
        return np.exp(-0.5 * d * d / L)

    cov = rbf_np(t_in, t_in) + np.eye(C) * SIGMA
    return np.linalg.solve(cov, rbf_np(t_in, t_pred)).T   # [P, C] float64


def _prep_constants():
    km = _k_mult()
    energy = (km * km).sum(axis=0)
    order = np.argsort(energy)
    cols8 = np.sort(order[:N8])       # fp8 columns, natural order
    cols16 = np.sort(order[N8:])      # fp16 columns

    # fp16 weights: [128, 2*P]; block 0 = cols16[0:128], block 1 (p<64) =
    # cols16[128:192]
    km16 = np.zeros((128, 2 * P), np.float16)
    km16[:, :P] = km[:, cols16[:128]].T
    km16[:64, P:] = km[:, cols16[128:]].T

    # fp8 weights with balanced per-column scale, subtile q = cols8[q*128+p]
    g = np.sqrt(np.abs(km[:, cols8]).max(axis=0))          # [N8]
    km8 = (km[:, cols8] / g).astype(F8)                    # [P, N8]
    km8_dev = np.zeros((128, 2, P), F8)
    for q in range(2):
        km8_dev[:, q, :] = km8[:, q * 128 : (q + 1) * 128].T
    return cols16, cols8, g.astype(np.float32), km16, km8_dev.reshape(128, 2 * P)


COLS16, COLS8, G8, KM16_DEV, KM8_DEV = _prep_constants()

_NC = None


def _build():
    nc = bacc.Bacc()
    # per pair: [128, 2*D] fp16, col half h = batch 2p+h's K-tile-0 rows
    zj0 = nc.dram_tensor("zj0", [NPAIR * 128, 2 * D], mybir.dt.float16,
                         kind="ExternalInput")
    # per pair: [64, 2*D] fp16, col half h = batch 2p+h's K-tile-1 rows
    zj1 = nc.dram_tensor("zj1", [NPAIR * 64, 2 * D], mybir.dt.float16,
                         kind="ExternalInput")
    # per pair: [128, 4*D] fp8, cols [h*2D + q*D : ...] = batch h, subtile q
    z8 = nc.dram_tensor("z8", [NPAIR * 128, 4 * D], mybir.dt.float8e4,
                        kind="ExternalInput")
    km16 = nc.dram_tensor("km16", [128, 2 * P], mybir.dt.float16,
                          kind="ExternalInput")
    km8 = nc.dram_tensor("km8", [128, 2 * P], mybir.dt.float8e4,
                         kind="ExternalInput")
    out = nc.dram_tensor("out", [BPC * P, D], mybir.dt.float16,
                         kind="ExternalOutput")

    with tile.TileContext(nc) as tc, ExitStack() as ctx:
        kpool = ctx.enter_context(tc.tile_pool(name="km", bufs=1))
        wpool = ctx.enter_context(tc.tile_pool(name="warm", bufs=1))
        z0pool = ctx.enter_context(tc.tile_pool(name="zj0", bufs=3))
        z1pool = ctx.enter_context(tc.tile_pool(name="zj1", bufs=3))
        z8pool = ctx.enter_context(tc.tile_pool(name="z8", bufs=3))
        opool = ctx.enter_context(tc.tile_pool(name="o", bufs=3))
        ppool = ctx.enter_context(tc.tile_pool(name="ps", bufs=7, space="PSUM"))
        wppool = ctx.enter_context(tc.tile_pool(name="wps", bufs=1, space="PSUM"))

        # Warm-up: garbage matmuls with no data dependencies. They run while
        # the first tiles are still in DMA flight, so the PE power governor
        # sees sustained activity early and lifts the p-state cap before the
        # real matmuls start. Results land in a scratch PSUM tile, never read.
        warm = wpool.tile([128, 576], mybir.dt.float16)
        nc.gpsimd.memset(warm[:, :], 1.0)
        wps = wppool.tile([P, 512], mybir.dt.float32)
        for _ in range(NWARM):
            nc.tensor.matmul(wps[:, :], warm[:, :P], warm[:, P : P + 512],
                             start=True, stop=True)

        km16_sb = kpool.tile([128, 2 * P], mybir.dt.float16)
        km8_sb = kpool.tile([128, 2, P], mybir.dt.float8e4)
        nc.sync.dma_start(km8_sb[:, :, :], km8[:, :])
        nc.scalar.dma_start(km16_sb[:, :], km16[:, :])

        for pr in range(NPAIR):
            e0 = nc.sync if pr % 2 == 0 else nc.scalar
            e1 = nc.scalar if pr % 2 == 0 else nc.sync

            z0t = z0pool.tile([128, 2 * D], mybir.dt.float16,
                              name=f"z0_{pr}", tag="z0")
            e0.dma_start(z0t[:, :], zj0[pr * 128 : (pr + 1) * 128, :])
            z1t = z1pool.tile([64, 2 * D], mybir.dt.float16,
                              name=f"z1_{pr}", tag="z1")
            e0.dma_start(z1t[:, :], zj1[pr * 64 : (pr + 1) * 64, :])
            z8t = z8pool.tile([128, 4 * D], mybir.dt.float8e4,
                              name=f"z8_{pr}", tag="z8")
            e1.dma_start(z8t[:, :], z8[pr * 128 : (pr + 1) * 128, :])

            out_sb = opool.tile([128, D], mybir.dt.float16,
                                name=f"o_{pr}", tag="o")
            for h in range(2):
                for n in range(2):
                    ps = ppool.tile([P, 512], mybir.dt.float32)
                    col = h * D + n * 512
                    nc.tensor.matmul(ps[:, :], km16_sb[:, :P],
                                     z0t[:, col : col + 512],
                                     start=True, stop=False)
                    nc.tensor.matmul(ps[:, :], km16_sb[:64, P : 2 * P],
                                     z1t[:, col : col + 512],
                                     start=False, stop=False)
                    for q in range(2):
                        col8 = h * 2 * D + q * D + n * 512
                        nc.tensor.matmul(ps[:, :], km8_sb[:, q, :],
                                         z8t[:, col8 : col8 + 512],
                                         start=False, stop=(q == 1))
                    nc.vector.tensor_copy(
                        out_sb[h * P : (h + 1) * P, n * 512 : (n + 1) * 512],
                        ps[:, :])
            eo = nc.sync if pr % 2 == 1 else nc.scalar
            eo.dma_start(out[pr * 128 : (pr + 1) * 128, :], out_sb[:, :])

    nc.finalize()
    return nc


def kernel(z_enc: np.ndarray, _trace: bool = False):
    global _NC
    z_enc = np.asarray(z_enc, dtype=np.float32)
    if _NC is None:
        _NC = _build()

    zc = z_enc[:, :C, :]
    z16 = np.ascontiguousarray(zc[:, COLS16, :]).astype(np.float16)  # [B,192,D]
    # pair-pack: [B/2, 2, rows, D] -> [B/2, rows, 2, D]
    zj0 = np.ascontiguousarray(
        z16[:, :128, :].reshape(B // 2, 2, 128, D).transpose(0, 2, 1, 3)
    ).reshape(B // 2 * 128, 2 * D)
    zj1 = np.ascontiguousarray(
        z16[:, 128:, :].reshape(B // 2, 2, 64, D).transpose(0, 2, 1, 3)
    ).reshape(B // 2 * 64, 2 * D)

    z8f = np.ascontiguousarray(zc[:, COLS8, :]) * G8[None, :, None]  # [B,256,D]
    # [B, 2(q), 128, D] -> pairs [B/2, 128, 2(h), 2(q), D]
    z8 = np.ascontiguousarray(
        z8f.reshape(B // 2, 2, 2, 128, D).transpose(0, 3, 1, 2, 4)
    ).astype(F8).reshape(B // 2 * 128, 4 * D)

    r0, r1, r8 = NPAIR * 128, NPAIR * 64, NPAIR * 128
    in_maps = [
        {
            "zj0": zj0[i * r0 : (i + 1) * r0],
            "zj1": zj1[i * r1 : (i + 1) * r1],
            "z8": z8[i * r8 : (i + 1) * r8],
            "km16": KM16_DEV,
            "km8": KM8_DEV,
        }
        for i in range(N_CORES)
    ]

    res = run_bass_kernel_spmd(_NC, in_maps, core_ids=list(range(N_CORES)),
                               trace=_trace)
    out = np.concatenate(
        [r["out"].reshape(BPC, P, D) for r in res.results], axis=0
    ).astype(np.float32)
    if _trace:
        return out, res
    return out


# revision 12
# speedup vs baseline: 1.1276x; 1.0912x over previous
"""GP prediction kernel for Trainium2 (8 NeuronCores, data-parallel over batch).

Computes z_pred[b, p, d] = sum_c k_mult[p, c] * z_enc[b, c, d] where k_mult
is the [64, 448] GP weight matrix k_pred.T @ inv(cov + sigma*I). k_mult
depends only on compile-time constants, so it is precomputed on host; the
device work is a batched [64,448] @ [448,1024] matmul, sharded 8 batches
per core.

Mixed precision against the 2e-2 correctness gate: the contraction is
split by column energy of k_mult. The 192 highest-energy context steps
travel in fp16 (K-tiles of 128 + 64 rows); the 256 lowest-energy steps
(2.6% of weight energy) travel as fp8-e4m3 with a per-column balanced
scale g_c = sqrt(max|km_c|) folded into both operands, consumed by two
plain fp8 matmuls (NOT DoubleRow - that mode pins the PE power governor
to the half-speed p-state for the whole kernel). End-to-end error ~6e-3.

Batches are packed in pairs per SBUF tile to halve the DMA instruction
count, and a burst of garbage warm-up matmuls runs while the first real
tiles are still in flight so the PE power governor's full-speed grant
(which needs sustained PE activity) arrives before the real matmuls do.
"""
import numpy as np
from contextlib import ExitStack

import concourse.bacc as bacc
import concourse.tile as tile
from concourse import mybir
from concourse.bass_utils import run_bass_kernel_spmd

# Problem constants (hardcoded per harness contract).
B, T, D = 64, 512, 1024
P = 64                 # N_PREDICTORS
C = T - P              # 448 context timesteps
L, SIGMA, TIMESCALE = 0.01, 0.01, 0.3
N_CORES = 8
BPC = B // N_CORES     # batches per core
NPAIR = BPC // 2       # batch pairs per core

N8 = 256               # low-energy columns in fp8 (2 K-tiles of 128)
N16 = C - N8           # high-energy columns in fp16 (128 + 64 K-tiles)
NWARM = 8              # garbage matmuls to pull the PE p-state grant early

F8 = mybir.dt.np(mybir.dt.float8e4)   # ml_dtypes.float8_e4m3


def _k_mult() -> np.ndarray:
    """[P, C] GP weight matrix, solved in float64 on host."""
    t = np.linspace(0.0, 1.0, T)
    t_in = t[:C] * TIMESCALE
    t_pred = t[C:] * TIMESCALE

    def rbf_np(x, y):
        d = x[:, None] - y[None, :]
        return np.exp(-0.5 * d * d / L)

    cov = rbf_np(t_in, t_in) + np.eye(C) * SIGMA
    return np.linalg.solve(cov, rbf_np(t_in, t_pred)).T   # [P, C] float64


def _prep_constants():
    km = _k_mult()
    energy = (km * km).sum(axis=0)
    order = np.argsort(energy)
    cols8 = np.sort(order[:N8])       # fp8 columns, natural order
    cols16 = np.sort(order[N8:])      # fp16 columns

    # fp16 weights: [128, 2*P]; block 0 = cols16[0:128], block 1 (p<64) =
    # cols16[128:192]
    km16 = np.zeros((128, 2 * P), np.float16)
    km16[:, :P] = km[:, cols16[:128]].T
    km16[:64, P:] = km[:, cols16[128:]].T

    # fp8 weights with balanced per-column scale, subtile q = cols8[q*128+p]
    g = np.sqrt(np.abs(km[:, cols8]).max(axis=0))          # [N8]
    km8 = (km[:, cols8] / g).astype(F8)                    # [P, N8]
    km8_dev = np.zeros((128, 2, P), F8)
    for q in range(2):
        km8_dev[:, q, :] = km8[:, q * 128 : (q + 1) * 128].T
    return cols16, cols8, g.astype(np.float32), km16, km8_dev.reshape(128, 2 * P)


COLS16, COLS8, G8, KM16_DEV, KM8_DEV = _prep_constants()

_NC = None


def _build():
    nc = bacc.Bacc()
    # per pair: [128, 2*D] fp16, col half h = batch 2p+h's K-tile-0 rows
    zj0 = nc.dram_tensor("zj0", [NPAIR * 128, 2 * D], mybir.dt.float16,
                         kind="ExternalInput")
    # per pair: [64, 2*D] fp16, col half h = batch 2p+h's K-tile-1 rows
    zj1 = nc.dram_tensor("zj1", [NPAIR * 64, 2 * D], mybir.dt.float16,
                         kind="ExternalInput")
    # per pair: [128, 4*D] fp8, cols [h*2D + q*D : ...] = batch h, subtile q
    z8 = nc.dram_tensor("z8", [NPAIR * 128, 4 * D], mybir.dt.float8e4,
                        kind="ExternalInput")
    km16 = nc.dram_tensor("km16", [128, 2 * P], mybir.dt.float16,
                          kind="ExternalInput")
    km8 = nc.dram_tensor("km8", [128, 2 * P], mybir.dt.float8e4,
                         kind="ExternalInput")
    out = nc.dram_tensor("out", [BPC * P, D], mybir.dt.float16,
                         kind="ExternalOutput")

    with tile.TileContext(nc) as tc, ExitStack() as ctx:
        kpool = ctx.enter_context(tc.tile_pool(name="km", bufs=1))
        wpool = ctx.enter_context(tc.tile_pool(name="warm", bufs=1))
        z0pool = ctx.enter_context(tc.tile_pool(name="zj0", bufs=3))
        z1pool = ctx.enter_context(tc.tile_pool(name="zj1", bufs=3))
        z8pool = ctx.enter_context(tc.tile_pool(name="z8", bufs=3))
        opool = ctx.enter_context(tc.tile_pool(name="o", bufs=3))
        ppool = ctx.enter_context(tc.tile_pool(name="ps", bufs=7, space="PSUM"))
        wppool = ctx.enter_context(tc.tile_pool(name="wps", bufs=1, space="PSUM"))

        # Warm-up: garbage matmuls with no data dependencies. They run while
        # the first tiles are still in DMA flight, so the PE power governor
        # sees sustained activity early and lifts the p-state cap before the
        # real matmuls start. Results land in a scratch PSUM tile, never read.
        warm = wpool.tile([128, 576], mybir.dt.float16)
        nc.gpsimd.memset(warm[:, :], 1.0)
        wps = wppool.tile([P, 512], mybir.dt.float32)
        for _ in range(NWARM):
            nc.tensor.matmul(wps[:, :], warm[:, :P], warm[:, P : P + 512],
                             start=True, stop=True)

        km16_sb = kpool.tile([128, 2 * P], mybir.dt.float16)
        km8_sb = kpool.tile([128, 2, P], mybir.dt.float8e4)
        nc.sync.dma_start(km8_sb[:, :, :], km8[:, :])
        nc.scalar.dma_start(km16_sb[:, :], km16[:, :])

        for pr in range(NPAIR):
            e0 = nc.sync if pr % 2 == 0 else nc.scalar
            e1 = nc.scalar if pr % 2 == 0 else nc.sync

            z0t = z0pool.tile([128, 2 * D], mybir.dt.float16,
                              name=f"z0_{pr}", tag="z0")
            e0.dma_start(z0t[:, :], zj0[pr * 128 : (pr + 1) * 128, :])
            z1t = z1pool.tile([64, 2 * D], mybir.dt.float16,
                              name=f"z1_{pr}", tag="z1")
            e0.dma_start(z1t[:, :], zj1[pr * 64 : (pr + 1) * 64, :])
            z8t = z8pool.tile([128, 4 * D], mybir.dt.float8e4,
                              name=f"z8_{pr}", tag="z8")
            e1.dma_start(z8t[:, :], z8[pr * 128 : (pr + 1) * 128, :])

            out_sb = opool.tile([128, D], mybir.dt.float16,
                                name=f"o_{pr}", tag="o")
            # j-outer: consecutive matmuls share stationary weights across
            # the 4 (h, n) PSUM groups of the pair
            ps = [ppool.tile([P, 512], mybir.dt.float32, name=f"ps{pr}_{g}",
                             tag="ps") for g in range(4)]
            units = (
                [(km16_sb[:, :P],
                  lambda h, n: z0t[:, h * D + n * 512 : h * D + (n + 1) * 512])]
                + [(km16_sb[:64, P : 2 * P],
                    lambda h, n: z1t[:, h * D + n * 512 : h * D + (n + 1) * 512])]
                + [(km8_sb[:, q, :],
                    lambda h, n, q=q: z8t[:, h * 2 * D + q * D + n * 512 :
                                          h * 2 * D + q * D + (n + 1) * 512])
                   for q in range(2)]
            )
            for j, (w, rhs_of) in enumerate(units):
                for g in range(4):
                    h, n = g // 2, g % 2
                    nc.tensor.matmul(ps[g][:, :], w, rhs_of(h, n),
                                     start=(j == 0), stop=(j == 3),
                                     skip_group_check=True)
            for g in range(4):
                h, n = g // 2, g % 2
                nc.vector.tensor_copy(
                    out_sb[h * P : (h + 1) * P, n * 512 : (n + 1) * 512],
                    ps[g][:, :])
            eo = nc.sync if pr % 2 == 1 else nc.scalar
            for h in range(2):
                eo.dma_start(out[pr * 128 + h * P : pr * 128 + (h + 1) * P, :],
                             out_sb[h * P : (h + 1) * P, :])

    nc.finalize()
    return nc


def kernel(z_enc: np.ndarray, _trace: bool = False):
    global _NC
    z_enc = np.asarray(z_enc, dtype=np.float32)
    if _NC is None:
        _NC = _build()

    zc = z_enc[:, :C, :]
    z16 = np.ascontiguousarray(zc[:, COLS16, :]).astype(np.float16)  # [B,192,D]
    # pair-pack: [B/2, 2, rows, D] -> [B/2, rows, 2, D]
    zj0 = np.ascontiguousarray(
        z16[:, :128, :].reshape(B // 2, 2, 128, D).transpose(0, 2, 1, 3)
    ).reshape(B // 2 * 128, 2 * D)
    zj1 = np.ascontiguousarray(
        z16[:, 128:, :].reshape(B // 2, 2, 64, D).transpose(0, 2, 1, 3)
    ).reshape(B // 2 * 64, 2 * D)

    z8f = np.ascontiguousarray(zc[:, COLS8, :]) * G8[None, :, None]  # [B,256,D]
    # [B, 2(q), 128, D] -> pairs [B/2, 128, 2(h), 2(q), D]
    z8 = np.ascontiguousarray(
        z8f.reshape(B // 2, 2, 2, 128, D).transpose(0, 3, 1, 2, 4)
    ).astype(F8).reshape(B // 2 * 128, 4 * D)

    r0, r1, r8 = NPAIR * 128, NPAIR * 64, NPAIR * 128
    in_maps = [
        {
            "zj0": zj0[i * r0 : (i + 1) * r0],
            "zj1": zj1[i * r1 : (i + 1) * r1],
            "z8": z8[i * r8 : (i + 1) * r8],
            "km16": KM16_DEV,
            "km8": KM8_DEV,
        }
        for i in range(N_CORES)
    ]

    res = run_bass_kernel_spmd(_NC, in_maps, core_ids=list(range(N_CORES)),
                               trace=_trace)
    out = np.concatenate(
        [r["out"].reshape(BPC, P, D) for r in res.results], axis=0
    ).astype(np.float32)
    if _trace:
        return out, res
    return out


# revision 13
# speedup vs baseline: 1.1286x; 1.0009x over previous
"""GP prediction kernel for Trainium2 (8 NeuronCores, data-parallel over batch).

Computes z_pred[b, p, d] = sum_c k_mult[p, c] * z_enc[b, c, d] where k_mult
is the [64, 448] GP weight matrix k_pred.T @ inv(cov + sigma*I). k_mult
depends only on compile-time constants, so it is precomputed on host; the
device work is a batched [64,448] @ [448,1024] matmul, sharded 8 batches
per core.

Mixed precision against the 2e-2 correctness gate: the contraction is
split by column energy of k_mult. The 192 highest-energy context steps
travel in fp16 (K-tiles of 128 + 64 rows); the 256 lowest-energy steps
(2.6% of weight energy) travel as fp8-e4m3 with a per-column balanced
scale g_c = sqrt(max|km_c|) folded into both operands, consumed by two
plain fp8 matmuls (NOT DoubleRow - that mode pins the PE power governor
to the half-speed p-state for the whole kernel). End-to-end error ~6e-3.

Batches are packed in pairs per SBUF tile to halve the DMA instruction
count, and a burst of garbage warm-up matmuls runs while the first real
tiles are still in flight so the PE power governor's full-speed grant
(which needs sustained PE activity) arrives before the real matmuls do.
"""
import numpy as np
from contextlib import ExitStack

import concourse.bacc as bacc
import concourse.tile as tile
from concourse import mybir
from concourse.bass_utils import run_bass_kernel_spmd

# Problem constants (hardcoded per harness contract).
B, T, D = 64, 512, 1024
P = 64                 # N_PREDICTORS
C = T - P              # 448 context timesteps
L, SIGMA, TIMESCALE = 0.01, 0.01, 0.3
N_CORES = 8
BPC = B // N_CORES     # batches per core
NPAIR = BPC // 2       # batch pairs per core

N8 = 256               # low-energy columns in fp8 (2 K-tiles of 128)
N16 = C - N8           # high-energy columns in fp16 (128 + 64 K-tiles)
NWARM = 8              # garbage matmuls to pull the PE p-state grant early

F8 = mybir.dt.np(mybir.dt.float8e4)   # ml_dtypes.float8_e4m3


def _k_mult() -> np.ndarray:
    """[P, C] GP weight matrix, solved in float64 on host."""
    t = np.linspace(0.0, 1.0, T)
    t_in = t[:C] * TIMESCALE
    t_pred = t[C:] * TIMESCALE

    def rbf_np(x, y):
        d = x[:, None] - y[None, :]
        return np.exp(-0.5 * d * d / L)

    cov = rbf_np(t_in, t_in) + np.eye(C) * SIGMA
    return np.linalg.solve(cov, rbf_np(t_in, t_pred)).T   # [P, C] float64


def _prep_constants():
    km = _k_mult()
    energy = (km * km).sum(axis=0)
    order = np.argsort(energy)
    cols8 = np.sort(order[:N8])       # fp8 columns, natural order
    cols16 = np.sort(order[N8:])      # fp16 columns

    # fp16 weights: [128, 2*P]; block 0 = cols16[0:128], block 1 (p<64) =
    # cols16[128:192]
    km16 = np.zeros((128, 2 * P), np.float16)
    km16[:, :P] = km[:, cols16[:128]].T
    km16[:64, P:] = km[:, cols16[128:]].T

    # fp8 weights with balanced per-column scale, subtile q = cols8[q*128+p]
    g = np.sqrt(np.abs(km[:, cols8]).max(axis=0))          # [N8]
    km8 = (km[:, cols8] / g).astype(F8)                    # [P, N8]
    km8_dev = np.zeros((128, 2, P), F8)
    for q in range(2):
        km8_dev[:, q, :] = km8[:, q * 128 : (q + 1) * 128].T
    return cols16, cols8, g.astype(np.float32), km16, km8_dev.reshape(128, 2 * P)


COLS16, COLS8, G8, KM16_DEV, KM8_DEV = _prep_constants()

_NC = None


def _build():
    nc = bacc.Bacc()
    # per pair: [128, 2*D] fp16, col half h = batch 2p+h's K-tile-0 rows
    zj0 = nc.dram_tensor("zj0", [NPAIR * 128, 2 * D], mybir.dt.float16,
                         kind="ExternalInput")
    # per pair: [64, 2*D] fp16, col half h = batch 2p+h's K-tile-1 rows
    zj1 = nc.dram_tensor("zj1", [NPAIR * 64, 2 * D], mybir.dt.float16,
                         kind="ExternalInput")
    # per pair: [128, 4*D] fp8, cols [h*2D + q*D : ...] = batch h, subtile q
    z8 = nc.dram_tensor("z8", [NPAIR * 128, 4 * D], mybir.dt.float8e4,
                        kind="ExternalInput")
    km16 = nc.dram_tensor("km16", [128, 2 * P], mybir.dt.float16,
                          kind="ExternalInput")
    km8 = nc.dram_tensor("km8", [128, 2 * P], mybir.dt.float8e4,
                         kind="ExternalInput")
    out = nc.dram_tensor("out", [BPC * P, D], mybir.dt.float16,
                         kind="ExternalOutput")

    with tile.TileContext(nc) as tc, ExitStack() as ctx:
        kpool = ctx.enter_context(tc.tile_pool(name="km", bufs=1))
        wpool = ctx.enter_context(tc.tile_pool(name="warm", bufs=1))
        z0pool = ctx.enter_context(tc.tile_pool(name="zj0", bufs=3))
        z1pool = ctx.enter_context(tc.tile_pool(name="zj1", bufs=3))
        z8pool = ctx.enter_context(tc.tile_pool(name="z8", bufs=3))
        opool = ctx.enter_context(tc.tile_pool(name="o", bufs=3))
        ppool = ctx.enter_context(tc.tile_pool(name="ps", bufs=7, space="PSUM"))
        wppool = ctx.enter_context(tc.tile_pool(name="wps", bufs=1, space="PSUM"))

        # Warm-up: garbage matmuls with no data dependencies. They run while
        # the first tiles are still in DMA flight, so the PE power governor
        # sees sustained activity early and lifts the p-state cap before the
        # real matmuls start. Results land in a scratch PSUM tile, never read.
        warm = wpool.tile([128, 576], mybir.dt.float16)
        nc.gpsimd.memset(warm[:, :], 1.0)
        wps = wppool.tile([P, 512], mybir.dt.float32)
        for _ in range(NWARM):
            nc.tensor.matmul(wps[:, :], warm[:, :P], warm[:, P : P + 512],
                             start=True, stop=True)

        km16_sb = kpool.tile([128, 2 * P], mybir.dt.float16)
        km8_sb = kpool.tile([128, 2, P], mybir.dt.float8e4)
        nc.sync.dma_start(km8_sb[:, :, :], km8[:, :])
        nc.scalar.dma_start(km16_sb[:, :], km16[:, :])

        for pr in range(NPAIR):
            e0 = nc.sync if pr % 2 == 0 else nc.scalar
            e1 = nc.scalar if pr % 2 == 0 else nc.sync

            z0t = z0pool.tile([128, 2 * D], mybir.dt.float16,
                              name=f"z0_{pr}", tag="z0")
            e0.dma_start(z0t[:, :], zj0[pr * 128 : (pr + 1) * 128, :])
            z1t = z1pool.tile([64, 2 * D], mybir.dt.float16,
                              name=f"z1_{pr}", tag="z1")
            e0.dma_start(z1t[:, :], zj1[pr * 64 : (pr + 1) * 64, :])
            z8t = z8pool.tile([128, 4 * D], mybir.dt.float8e4,
                              name=f"z8_{pr}", tag="z8")
            e1.dma_start(z8t[:, :], z8[pr * 128 : (pr + 1) * 128, :])

            out_sb = opool.tile([128, D], mybir.dt.float16,
                                name=f"o_{pr}", tag="o")
            # j-outer: consecutive matmuls share stationary weights across
            # the 4 (h, n) PSUM groups of the pair
            ps = [ppool.tile([P, 512], mybir.dt.float32, name=f"ps{pr}_{g}",
                             tag="ps") for g in range(4)]
            units = (
                [(km16_sb[:, :P],
                  lambda h, n: z0t[:, h * D + n * 512 : h * D + (n + 1) * 512])]
                + [(km16_sb[:64, P : 2 * P],
                    lambda h, n: z1t[:, h * D + n * 512 : h * D + (n + 1) * 512])]
                + [(km8_sb[:, q, :],
                    lambda h, n, q=q: z8t[:, h * 2 * D + q * D + n * 512 :
                                          h * 2 * D + q * D + (n + 1) * 512])
                   for q in range(2)]
            )
            for j, (w, rhs_of) in enumerate(units):
                for g in range(4):
                    h, n = g // 2, g % 2
                    nc.tensor.matmul(ps[g][:, :], w, rhs_of(h, n),
                                     start=(j == 0), stop=(j == 3),
                                     skip_group_check=True)
            for g in range(4):
                h, n = g // 2, g % 2
                dst = out_sb[h * P : (h + 1) * P, n * 512 : (n + 1) * 512]
                if n == 0:
                    nc.vector.tensor_copy(dst, ps[g][:, :])
                else:
                    nc.scalar.activation(dst, ps[g][:, :],
                                         mybir.ActivationFunctionType.Copy)
            eo = nc.sync if pr % 2 == 1 else nc.scalar
            for h in range(2):
                eo.dma_start(out[pr * 128 + h * P : pr * 128 + (h + 1) * P, :],
                             out_sb[h * P : (h + 1) * P, :])

    nc.finalize()
    return nc


def kernel(z_enc: np.ndarray, _trace: bool = False):
    global _NC
    z_enc = np.asarray(z_enc, dtype=np.float32)
    if _NC is None:
        _NC = _build()

    zc = z_enc[:, :C, :]
    z16 = np.ascontiguousarray(zc[:, COLS16, :]).astype(np.float16)  # [B,192,D]
    # pair-pack: [B/2, 2, rows, D] -> [B/2, rows, 2, D]
    zj0 = np.ascontiguousarray(
        z16[:, :128, :].reshape(B // 2, 2, 128, D).transpose(0, 2, 1, 3)
    ).reshape(B // 2 * 128, 2 * D)
    zj1 = np.ascontiguousarray(
        z16[:, 128:, :].reshape(B // 2, 2, 64, D).transpose(0, 2, 1, 3)
    ).reshape(B // 2 * 64, 2 * D)

    z8f = np.ascontiguousarray(zc[:, COLS8, :]) * G8[None, :, None]  # [B,256,D]
    # [B, 2(q), 128, D] -> pairs [B/2, 128, 2(h), 2(q), D]
    z8 = np.ascontiguousarray(
        z8f.reshape(B // 2, 2, 2, 128, D).transpose(0, 3, 1, 2, 4)
    ).astype(F8).reshape(B // 2 * 128, 4 * D)

    r0, r1, r8 = NPAIR * 128, NPAIR * 64, NPAIR * 128
    in_maps = [
        {
            "zj0": zj0[i * r0 : (i + 1) * r0],
            "zj1": zj1[i * r1 : (i + 1) * r1],
            "z8": z8[i * r8 : (i + 1) * r8],
            "km16": KM16_DEV,
            "km8": KM8_DEV,
        }
        for i in range(N_CORES)
    ]

    res = run_bass_kernel_spmd(_NC, in_maps, core_ids=list(range(N_CORES)),
                               trace=_trace)
    out = np.concatenate(
        [r["out"].reshape(BPC, P, D) for r in res.results], axis=0
    ).astype(np.float32)
    if _trace:
        return out, res
    return out


# revision 14
# speedup vs baseline: 1.2328x; 1.0924x over previous
"""GP prediction kernel for Trainium2 (8 NeuronCores, data-parallel over batch).

Computes z_pred[b, p, d] = sum_c k_mult[p, c] * z_enc[b, c, d] where k_mult
is the [64, 448] GP weight matrix k_pred.T @ inv(cov + sigma*I). k_mult
depends only on compile-time constants, so it is precomputed on host; the
device work is a batched [64,448] @ [448,1024] matmul, sharded 8 batches
per core.

Mixed precision against the 2e-2 correctness gate: the contraction is
split by column energy of k_mult. The 128 highest-energy context steps
travel in fp16; the 320 lowest-energy steps (5.7% of weight energy)
travel as fp8-e4m3 with a per-column balanced scale g_c = sqrt(max|km_c|)
folded into both operands, consumed by plain fp8 matmuls (NOT DoubleRow -
that mode pins the PE power governor to the half-speed p-state). The fp8
majority also lowers PE power draw, which lengthens the governor's
full-speed grant windows. End-to-end error ~9e-3.

Batches are packed in pairs per SBUF tile to halve DMA instruction
count. Matmuls are ordered weights-outer so consecutive matmuls share
stationary weights and pipeline at the 1 column/cycle streaming floor.
A burst of garbage warm-up matmuls runs while the first real tiles are
in DMA flight so the PE power governor's full-speed grant arrives
before the real matmuls do.
"""
import numpy as np
from contextlib import ExitStack

import concourse.bacc as bacc
import concourse.tile as tile
from concourse import mybir
from concourse.bass_utils import run_bass_kernel_spmd

# Problem constants (hardcoded per harness contract).
B, T, D = 64, 512, 1024
P = 64                 # N_PREDICTORS
C = T - P              # 448 context timesteps
L, SIGMA, TIMESCALE = 0.01, 0.01, 0.3
N_CORES = 8
BPC = B // N_CORES     # batches per core
NPAIR = BPC // 2       # batch pairs per core

N8 = 320               # low-energy columns in fp8 (K-tiles 128+128+64)
N16 = C - N8           # high-energy columns in fp16 (one K-tile of 128)
NWARM = 8              # garbage matmuls to pull the PE p-state grant early

F8 = mybir.dt.np(mybir.dt.float8e4)   # ml_dtypes.float8_e4m3


def _k_mult() -> np.ndarray:
    """[P, C] GP weight matrix, solved in float64 on host."""
    t = np.linspace(0.0, 1.0, T)
    t_in = t[:C] * TIMESCALE
    t_pred = t[C:] * TIMESCALE

    def rbf_np(x, y):
        d = x[:, None] - y[None, :]
        return np.exp(-0.5 * d * d / L)

    cov = rbf_np(t_in, t_in) + np.eye(C) * SIGMA
    return np.linalg.solve(cov, rbf_np(t_in, t_pred)).T   # [P, C] float64


def _prep_constants():
    km = _k_mult()
    energy = (km * km).sum(axis=0)
    order = np.argsort(energy)
    cols8 = np.sort(order[:N8])       # fp8 columns, natural order
    cols16 = np.sort(order[N8:])      # fp16 columns (128)

    km16 = np.ascontiguousarray(km[:, cols16].T.astype(np.float16))  # [128, P]

    # fp8 weights with balanced per-column scale; subtiles q=0,1 full 128
    # rows (cols8[q*128+p]), subtile 2 the last 64 rows (p<64)
    g = np.sqrt(np.abs(km[:, cols8]).max(axis=0))          # [N8]
    km8 = (km[:, cols8] / g).astype(F8)                    # [P, N8]
    km8_dev = np.zeros((128, 3 * P), F8)
    for q in range(2):
        km8_dev[:, q * P : (q + 1) * P] = km8[:, q * 128 : (q + 1) * 128].T
    km8_dev[:64, 2 * P : 3 * P] = km8[:, 256:320].T
    return cols8, cols16, g.astype(np.float32), km16, km8_dev


COLS8, COLS16, G8, KM16_DEV, KM8_DEV = _prep_constants()

_NC = None


def _build():
    nc = bacc.Bacc()
    # per pair: [128, 2*D] fp16, col half h = batch 2p+h's fp16 K-tile rows
    zj0 = nc.dram_tensor("zj0", [NPAIR * 128, 2 * D], mybir.dt.float16,
                         kind="ExternalInput")
    # per pair: [128, 4*D] fp8, cols [h*2D + q*D : ...] = batch h, subtile q
    z8 = nc.dram_tensor("z8", [NPAIR * 128, 4 * D], mybir.dt.float8e4,
                        kind="ExternalInput")
    # per pair: [64, 2*D] fp8, col half h = batch h's subtile-2 rows
    z82 = nc.dram_tensor("z82", [NPAIR * 64, 2 * D], mybir.dt.float8e4,
                         kind="ExternalInput")
    km16 = nc.dram_tensor("km16", [128, P], mybir.dt.float16,
                          kind="ExternalInput")
    km8 = nc.dram_tensor("km8", [128, 3 * P], mybir.dt.float8e4,
                         kind="ExternalInput")
    out = nc.dram_tensor("out", [BPC * P, D], mybir.dt.float16,
                         kind="ExternalOutput")

    with tile.TileContext(nc) as tc, ExitStack() as ctx:
        kpool = ctx.enter_context(tc.tile_pool(name="km", bufs=1))
        wpool = ctx.enter_context(tc.tile_pool(name="warm", bufs=1))
        z0pool = ctx.enter_context(tc.tile_pool(name="zj0", bufs=3))
        z8pool = ctx.enter_context(tc.tile_pool(name="z8", bufs=3))
        z82pool = ctx.enter_context(tc.tile_pool(name="z82", bufs=3))
        opool = ctx.enter_context(tc.tile_pool(name="o", bufs=3))
        ppool = ctx.enter_context(tc.tile_pool(name="ps", bufs=7, space="PSUM"))
        wppool = ctx.enter_context(tc.tile_pool(name="wps", bufs=1, space="PSUM"))

        # Warm-up: garbage matmuls with no data dependencies, running while
        # the first tiles are in DMA flight, so the PE power governor lifts
        # the p-state cap before the real matmuls start.
        warm = wpool.tile([128, 576], mybir.dt.float16)
        nc.gpsimd.memset(warm[:, :], 1.0)
        wps = wppool.tile([P, 512], mybir.dt.float32)
        for _ in range(NWARM):
            nc.tensor.matmul(wps[:, :], warm[:, :P], warm[:, P : P + 512],
                             start=True, stop=True)

        km16_sb = kpool.tile([128, P], mybir.dt.float16)
        km8_sb = kpool.tile([128, 3 * P], mybir.dt.float8e4)
        nc.sync.dma_start(km8_sb[:, :], km8[:, :])
        nc.scalar.dma_start(km16_sb[:, :], km16[:, :])

        for pr in range(NPAIR):
            e0 = nc.sync if pr % 2 == 0 else nc.scalar
            e1 = nc.scalar if pr % 2 == 0 else nc.sync

            z0t = z0pool.tile([128, 2 * D], mybir.dt.float16,
                              name=f"z0_{pr}", tag="z0")
            e0.dma_start(z0t[:, :], zj0[pr * 128 : (pr + 1) * 128, :])
            z82t = z82pool.tile([64, 2 * D], mybir.dt.float8e4,
                                name=f"z82_{pr}", tag="z82")
            e0.dma_start(z82t[:, :], z82[pr * 64 : (pr + 1) * 64, :])
            z8t = z8pool.tile([128, 4 * D], mybir.dt.float8e4,
                              name=f"z8_{pr}", tag="z8")
            e1.dma_start(z8t[:, :], z8[pr * 128 : (pr + 1) * 128, :])

            out_sb = opool.tile([128, D], mybir.dt.float16,
                                name=f"o_{pr}", tag="o")
            # weights-outer: consecutive matmuls share stationary weights
            # across the 4 (h, n) PSUM groups of the pair
            ps = [ppool.tile([P, 512], mybir.dt.float32, name=f"ps{pr}_{g}",
                             tag="ps") for g in range(4)]
            units = (
                [(km16_sb[:, :],
                  lambda h, n: z0t[:, h * D + n * 512 : h * D + (n + 1) * 512])]
                + [(km8_sb[:, q * P : (q + 1) * P],
                    lambda h, n, q=q: z8t[:, h * 2 * D + q * D + n * 512 :
                                          h * 2 * D + q * D + (n + 1) * 512])
                   for q in range(2)]
                + [(km8_sb[:64, 2 * P : 3 * P],
                    lambda h, n: z82t[:, h * D + n * 512 : h * D + (n + 1) * 512])]
            )
            for j, (w, rhs_of) in enumerate(units):
                for g in range(4):
                    h, n = g // 2, g % 2
                    nc.tensor.matmul(ps[g][:, :], w, rhs_of(h, n),
                                     start=(j == 0), stop=(j == 3),
                                     skip_group_check=True)
            for g in range(4):
                h, n = g // 2, g % 2
                dst = out_sb[h * P : (h + 1) * P, n * 512 : (n + 1) * 512]
                if n == 0:
                    nc.vector.tensor_copy(dst, ps[g][:, :])
                else:
                    nc.scalar.activation(dst, ps[g][:, :],
                                         mybir.ActivationFunctionType.Copy)
            eo = nc.sync if pr % 2 == 1 else nc.scalar
            for h in range(2):
                eo.dma_start(out[pr * 128 + h * P : pr * 128 + (h + 1) * P, :],
                             out_sb[h * P : (h + 1) * P, :])

    nc.finalize()
    return nc


def kernel(z_enc: np.ndarray, _trace: bool = False):
    global _NC
    z_enc = np.asarray(z_enc, dtype=np.float32)
    if _NC is None:
        _NC = _build()

    zc = z_enc[:, :C, :]
    z16 = np.ascontiguousarray(zc[:, COLS16, :]).astype(np.float16)  # [B,128,D]
    zj0 = np.ascontiguousarray(
        z16.reshape(B // 2, 2, 128, D).transpose(0, 2, 1, 3)
    ).reshape(B // 2 * 128, 2 * D)

    z8f = np.ascontiguousarray(zc[:, COLS8, :]) * G8[None, :, None]  # [B,320,D]
    # subtiles 0,1: [B, 2(q), 128, D] -> pairs [B/2, 128, 2(h), 2(q), D]
    z8 = np.ascontiguousarray(
        z8f[:, :256].reshape(B // 2, 2, 2, 128, D).transpose(0, 3, 1, 2, 4)
    ).astype(F8).reshape(B // 2 * 128, 4 * D)
    # subtile 2: [B, 64, D] -> pairs [B/2, 64, 2(h), D]
    z82 = np.ascontiguousarray(
        z8f[:, 256:].reshape(B // 2, 2, 64, D).transpose(0, 2, 1, 3)
    ).astype(F8).reshape(B // 2 * 64, 2 * D)

    r0, r8, r82 = NPAIR * 128, NPAIR * 128, NPAIR * 64
    in_maps = [
        {
            "zj0": zj0[i * r0 : (i + 1) * r0],
            "z8": z8[i * r8 : (i + 1) * r8],
            "z82": z82[i * r82 : (i + 1) * r82],
            "km16": KM16_DEV,
            "km8": KM8_DEV,
        }
        for i in range(N_CORES)
    ]

    res = run_bass_kernel_spmd(_NC, in_maps, core_ids=list(range(N_CORES)),
                               trace=_trace)
    out = np.concatenate(
        [r["out"].reshape(BPC, P, D) for r in res.results], axis=0
    ).astype(np.float32)
    if _trace:
        return out, res
    return out
